# revision 24
# baseline (speedup 1.0000x reference)
"""LoRA QKV with slot routing on 8 TRN2 cores — sorted-token sparse variant.

Host sorts tokens by LoRA slot, so each core's 1024-token shard touches at
most 4 slots (uniform routing over 8 slots; a 1024-token window of the sorted
order can't span more). Per-core LoRA state shrinks to 4 slots x 3 targets
x 16 ranks = 192 local ranks, padded to 256 so every matmul keeps K=M=128.
B matrices are zero-padded so each target group contracts over a full 128
partitions. The host un-permutes y afterwards.

Speed structure (measured on HW, 189us vs 212us all-bf16):
- fp8 DoubleRow: a DR matmul (2 fp8 k-planes) issues in one bf16 matmul's
  216ns slot -> each 15-instruction output chain covers 17 planes:
  DR(k0,k1 in e4m3, product-preserving scales x*S8 / W/S8) + 13 bf16 planes
  + DR(k15 | hm@B) where plane 1 pairs the on-device-quantized hm8 with B8.
  fp8 noise budget: 3 of 16 k-planes + hm/B at ~1.35e-2 rel err (gate 2e-2).
- stage A merges phase 1 (4 PSUM banks) with the first NA=4 j=0 chains
  (4 banks), k-paced on the input DMA stream, so the PE never idles long
  enough for the HAM clock gate to drop to 4/8.
- 10 warmup matmuls on a zeros tile bridge the HAM 1.2->2.4GHz ramp across
  the initial DMA-latency window.
- DMA: ~316GB/s/core aggregate, concurrent descriptors fair-share. Sync ring
  carries the k-paced (x, W-j0) stream; scalar carries a + stores. W j>=1
  halves are gated behind dummy w-pool generations (written after the mask
  multiply) so they cannot steal startup-window bandwidth; only k-chunks
  2..14 are fetched (0,1,15 ride fp8). Masks ship as fp8 (0/1 exact),
  y ships bf16 (host upcasts).
"""

import numpy as np
import ml_dtypes

import concourse.bass as bass
import concourse.bacc as bacc
import concourse.mybir as mybir
import concourse.tile as tile

HIDDEN = 2048
Q_SIZE = 2048
KV_SIZE = 512
OUT = Q_SIZE + 2 * KV_SIZE  # 3072
MAX_LORAS = 8
RANK = 16
T = 8192
N_CORES = 8
T_CORE = T // N_CORES  # 1024

P = 128
NT = T_CORE // P          # 8 token tiles per core
KC = HIDDEN // P          # 16 k-chunks
OJ = OUT // 512           # 6 output chunks of 512
NSLOT = 4                 # max distinct slots per sorted 1024-token window
LR = NSLOT * RANK         # 64 local ranks per target group
GRP = 2 * P               # 256 = [g0|g1|g2|zero-pad] packed as 2 groups of 128
GR = MAX_LORAS * RANK     # 128 (dense fallback rank count)
F32 = mybir.dt.float32
BF16 = mybir.dt.bfloat16
FP8 = mybir.dt.float8e4
FP8_NP = None  # ml_dtypes.float8_e4m3, resolved lazily
S8 = 0.25      # product-preserving fp8 scale: x*S8, (W,A)/S8

_NC_CACHE = {}


def build_nc():
    nc = bacc.Bacc("TRN2", target_bir_lowering=False, debug=False, num_devices=N_CORES)

    xT = nc.dram_tensor("xT", [HIDDEN, T_CORE], BF16, kind="ExternalInput").ap()
    wT = nc.dram_tensor("wT", [HIDDEN, OUT], BF16, kind="ExternalInput").ap()
    aT = nc.dram_tensor("aT", [HIDDEN, 192], BF16, kind="ExternalInput").ap()
    maskT = nc.dram_tensor("maskT", [GRP, T_CORE], FP8, kind="ExternalInput").ap()
    # fp8 (e4m3, TRN max-240 flavor) copies of k-chunks 0-1 for DoubleRow:
    # one DR matmul covers 2 k-planes in one bf16-matmul's issue slot
    # (216ns measured), saving one matmul per chain. Scales are product-
    # preserving (x*0.25, W*4) so DR output accumulates directly with the
    # bf16 chunks in PSUM.
    x8d = nc.dram_tensor("x8d", [P, 2 * T_CORE], FP8, kind="ExternalInput").ap()
    a8d = nc.dram_tensor("a8d", [P, 2 * GRP], FP8, kind="ExternalInput").ap()
    w8j0d = nc.dram_tensor("w8j0d", [P, 2 * 512], FP8, kind="ExternalInput").ap()
    w8rd = nc.dram_tensor("w8rd", [P, 2 * (OUT - 512)], FP8, kind="ExternalInput").ap()
    # second DR pair replaces (k15 bf16 MM + B MM): plane0 = x8ch15*w8ch15
    # (scales S8, 1/S8), plane1 = hm8*B8 (scales 1, 1) — per-plane products
    # are independently exact, so both accumulate into the same PSUM
    x8bd = nc.dram_tensor("x8bd", [P, 2 * T_CORE], FP8, kind="ExternalInput").ap()
    w8bj0d = nc.dram_tensor("w8bj0d", [P, 2 * 512], FP8, kind="ExternalInput").ap()
    w8brd = nc.dram_tensor("w8brd", [P, 2 * (OUT - 512)], FP8, kind="ExternalInput").ap()
    x8c15d = nc.dram_tensor("x8c15d", [P, T_CORE], FP8, kind="ExternalInput").ap()
    b8alld = nc.dram_tensor("b8alld", [P, 2 * OUT], FP8, kind="ExternalInput").ap()
    # y ships bf16 (host upcasts): halves store traffic + tail drain
    y = nc.dram_tensor("y", [T_CORE, OUT], BF16, kind="ExternalOutput").ap()

    NA = 4  # j0 chains merged into the k-paced stage A (PSUM: 4 hp + 4 ops)

    with tile.TileContext(nc) as tc:
        with (
            tc.tile_pool(name="xsb", bufs=1) as xpool,
            tc.tile_pool(name="asb", bufs=1) as apool,
            tc.tile_pool(name="wj0", bufs=1) as wj0pool,
            tc.tile_pool(name="bsb", bufs=1) as bpool,
            tc.tile_pool(name="msk", bufs=1) as mpool,
            tc.tile_pool(name="hm", bufs=1) as hmpool,
            tc.tile_pool(name="wrm", bufs=1) as wrmpool,
            tc.tile_pool(name="w", bufs=4) as wpool,
            tc.tile_pool(name="o", bufs=4) as opool,
            tc.tile_pool(name="hps", bufs=1, space="PSUM") as hpsum,
            tc.tile_pool(name="ops", bufs=4, space="PSUM") as opsum,
        ):
            xsb = xpool.tile([P, KC * T_CORE], BF16)   # free idx = k*T_CORE + t
            asb = apool.tile([P, KC * GRP], BF16)      # free idx = k*256 + lr
            wj0 = wj0pool.tile([P, KC * 512], BF16)    # W j=0, free idx = k*512+o
            aT3 = aT.rearrange("(c p) r -> p c r", p=P)
            xT3 = xT.rearrange("(c p) t -> p c t", p=P)
            wT3 = wT.rearrange("(c p) o -> p c o", p=P)
            asb3 = asb[:].rearrange("p (c r) -> p c r", c=KC)
            xsb3 = xsb[:].rearrange("p (c t) -> p c t", c=KC)
            wj03 = wj0[:].rearrange("p (c o) -> p c o", c=KC)
            x8sb = xpool.tile([P, 2 * T_CORE], FP8, tag="x8", name="x8sb")
            a8sb = apool.tile([P, 2 * GRP], FP8, tag="a8", name="a8sb")
            w8j0sb = wj0pool.tile([P, 2 * 512], FP8, tag="w8j0", name="w8j0sb")
            w8rsb = wj0pool.tile([P, 2 * (OUT - 512)], FP8, tag="w8r", name="w8rsb")
            x8bsb = xpool.tile([P, 2 * T_CORE], FP8, tag="x8b", name="x8bsb")
            w8bj0sb = wj0pool.tile([P, 2 * 512], FP8, tag="w8bj0", name="w8bj0sb")
            w8brsb = wj0pool.tile([P, 2 * (OUT - 512)], FP8, tag="w8br", name="w8brsb")
            x8b3 = x8bsb[:].rearrange("p (c t) -> p c t", c=2)
            w8bj03 = w8bj0sb[:].rearrange("p (c o) -> p c o", c=2)
            w8br3 = w8brsb[:].rearrange("p (c o) -> p c o", c=2)
            x83 = x8sb[:].rearrange("p (c t) -> p c t", c=2)
            a83 = a8sb[:].rearrange("p (c r) -> p c r", c=2)
            w8j03 = w8j0sb[:].rearrange("p (c o) -> p c o", c=2)
            w8r3 = w8rsb[:].rearrange("p (c o) -> p c o", c=2)
            # pair8: [x8ch15 | hm8 g0 | hm8 g1]; lhsT planes (0, 1+grp)
            pair8 = hmpool.tile([P, 3 * T_CORE], FP8, tag="pair8", name="pair8")
            pair83 = pair8[:].rearrange("p (c t) -> p c t", c=3)
            # b8all: 12 planes of 512: [w8ch15 j=0..5 | bq8 x4 | bk8 | bv8];
            # rhs planes (j, 6+bslot(j)) — always stride 6
            b8all = bpool.tile([P, 2 * OUT], FP8, tag="b8all", name="b8all")
            b8al3 = b8all[:].rearrange("p (c o) -> p c o", c=12)
            DR = mybir.MatmulPerfMode.DoubleRow
            # PE warmup: the HAM clock gate passes 4/8 pulses until ~3.4us of
            # sustained activity, so burn the initial DMA wait on dummy
            # matmuls (zeros tile; PSUM bank = hp1[0]'s earlier generation).
            wrm = wrmpool.tile([P, 640], BF16)
            nc.vector.memset(wrm[:], 0.0)
            # rank-pad columns of a (192:256 per chunk) must be finite zeros:
            # they feed hp2 garbage otherwise, and NaN*0 = NaN through the
            # mask multiply
            nc.gpsimd.memset(asb3[:, :, 192:256], 0.0)
            wps = hpsum.tile([P, 512], F32, tag="hp10", name="warm")
            for _ in range(10):
                nc.tensor.matmul(wps[:], lhsT=wrm[:, 0:128], rhs=wrm[:, 128:640],
                                 start=True, stop=True)
            # sync ring: interleaved (x, W j0) chunk pairs — everything stage A
            # consumes, in consumption order. scalar ring: a chunks + masks +
            # B + W j1/j2 halves (the first wpool generations have no
            # pool-rotation gate, so routing them here keeps them off the
            # x-critical sync queue; j>=3 halves are gated late and ride sync).
            nc.sync.dma_start(x8sb[:], x8d[:, :])
            nc.sync.dma_start(w8j0sb[:], w8j0d[:, :])
            nc.sync.dma_start(x8bsb[:], x8bd[:, :])
            nc.sync.dma_start(w8bj0sb[:], w8bj0d[:, :])
            for k in range(2, KC, 2):
                nc.sync.dma_start(xsb3[:, k:k + 2, :], xT3[:, k:k + 2, :])
                if k < 4:
                    continue  # j0 chains cover chunks 2,3 via the second DR
                if k < KC - 2:
                    nc.sync.dma_start(wj03[:, k:k + 2, :], wT3[:, k:k + 2, 0:512])
                else:
                    nc.sync.dma_start(wj03[:, k:k + 1, :], wT3[:, k:k + 1, 0:512])
            nc.scalar.dma_start(a8sb[:], a8d[:, :])
            for k in range(2, KC, 2):
                nc.scalar.dma_start(asb3[:, k:k + 2, 0:192], aT3[:, k:k + 2, :])
            # msk + B at the sync queue TAIL: the ring depth defers their
            # transfers until the stage-A inputs are mostly done (they're only
            # needed at ~36us; in the early soup they'd steal ~1.25MB of the
            # startup window's ~316GB/s)
            msk = mpool.tile([P, 2 * T_CORE], FP8)  # [:, 0:T]=grp1, [:, T:2T]=grp2
            nc.sync.dma_start(msk[:, 0:T_CORE], maskT[0:P, :])
            nc.sync.dma_start(msk[:, T_CORE:2 * T_CORE], maskT[P:GRP, :])
            nc.sync.dma_start(pair8[:, 0:T_CORE], x8c15d[:, :])
            nc.sync.dma_start(b8all[:], b8alld[:, :])
            nc.sync.dma_start(w8rsb[:], w8rd[:, :])
            nc.sync.dma_start(w8brsb[:], w8brd[:, :])

            # --- stage A, k-outer: phase-1 h chains (2 rank groups x 2 token
            # halves) + the first NA j=0 output chains. 7 MMs per chunk
            # (~1.5us) vs ~1.2us DMA per chunk: PE-bound, so the HAM clock
            # never drops mid-stream. ---
            hp1 = [hpsum.tile([P, 512], F32, tag=f"hp1{c}", name=f"hp1{c}")
                   for c in range(2)]
            hp2 = [hpsum.tile([P, 512], F32, tag=f"hp2{c}", name=f"hp2{c}")
                   for c in range(2)]
            opsA = [opsum.tile([P, 512], F32, tag="ops", name=f"opsA{c}")
                    for c in range(NA)]
            for grp, hp in ((0, hp1), (1, hp2)):
                for hh in range(2):
                    nc.tensor.matmul(
                        hp[hh][:], lhsT=a83[:, :, grp * P:(grp + 1) * P],
                        rhs=x83[:, :, hh * 512:(hh + 1) * 512],
                        start=True, stop=False, perf_mode=DR)
            for i in range(NA):
                nc.tensor.matmul(
                    opsA[i][:], lhsT=x83[:, :, i * P:(i + 1) * P],
                    rhs=w8j03[:, :, :],
                    start=True, stop=False, perf_mode=DR)
            for k in range(2, KC):
                for grp, hp in ((0, hp1), (1, hp2)):
                    lhsT = asb[:, k * GRP + grp * P: k * GRP + (grp + 1) * P]
                    for hh in range(2):
                        nc.tensor.matmul(
                            hp[hh][:], lhsT=lhsT,
                            rhs=xsb[:, k * T_CORE + hh * 512:
                                    k * T_CORE + (hh + 1) * 512],
                            start=False, stop=(k == KC - 1))
                if k == 3:
                    for i in range(NA):
                        nc.tensor.matmul(
                            opsA[i][:], lhsT=x8b3[:, :, i * P:(i + 1) * P],
                            rhs=w8bj03[:, :, :],
                            start=False, stop=False, perf_mode=DR)
                if k < 4 or k == KC - 1:
                    continue  # chunks 2,3 ride the DR; k15 pairs with B
                for i in range(NA):
                    nc.tensor.matmul(
                        opsA[i][:],
                        lhsT=xsb[:, k * T_CORE + i * P: k * T_CORE + (i + 1) * P],
                        rhs=wj03[:, k, :],
                        start=False, stop=False)
            for grp, hp in ((0, hp1), (1, hp2)):
                for hh in range(2):
                    nc.vector.tensor_tensor(
                        pair8[:, (1 + grp) * T_CORE + hh * 512:
                              (1 + grp) * T_CORE + (hh + 1) * 512],
                        hp[hh][:],
                        msk[:, grp * T_CORE + hh * 512: grp * T_CORE + (hh + 1) * 512],
                        op=mybir.AluOpType.mult)
            # Gate the W j>=1 fetches out of the startup window: burn the
            # first 4 "w"-pool generations on dummy tiles whose writer (a
            # 1-column DVE copy reading hm) only runs after the mask-mult, so
            # the real halves (gens 4+) cannot issue while stage A still
            # needs the DMA bandwidth. A 1MB descriptor only gets
            # ~70-100GB/s, but 4+ concurrent ones saturate ~316GB/s/core —
            # anything extra in the window starves the k-paced chunks.
            for g in range(4):
                wgate = wpool.tile([P, 8 * 512], BF16, tag="w", name="wgate")
                nc.vector.tensor_scalar_add(wgate[:, 0:1],
                                            pair8[:, T_CORE:T_CORE + 1], 0.0)

            def store(i, j, ops):
                osb = opool.tile([P, 512], BF16, tag="osb", name="osb")
                nc.scalar.copy(osb[:], ops[:])
                eng = nc.sync if (i + j) % 2 else nc.scalar
                eng.dma_start(
                    y[i * P:(i + 1) * P, j * 512:(j + 1) * 512], osb[:])

            # --- phase 2: y = x@W^T + hm@B, streamed over 512-wide o-chunks.
            # hmA rows: g0 ranks 0:64, g1 ranks 64:128 -> bq rows 64:128 are
            # zero and bk rows 0:64 are zero, so each B matmul contracts
            # K=128. j=0 finishes first: chain i=NA runs before the stage-A
            # B-matmuls so the DVE mask-multiply isn't on the PE critical
            # path. ---
            for j in range(OJ):
                grp = 1 if j == OJ - 1 else 0
                if j == 0:
                    def wslice(k):
                        return wj03[:, k, :]
                else:
                    wh = []
                    for half in range(2):
                        wsb = wpool.tile([P, 8 * 512], BF16, tag="w", name="wsb")
                        lo, hi = (4, 12) if half == 0 else (12, 15)
                        nc.sync.dma_start(
                            wsb[:, 0:(hi - lo) * 512],
                            wT3[:, lo:hi, j * 512:(j + 1) * 512])
                        wh.append(wsb)

                    def wslice(k, wh=wh):
                        h, base = (0, 4) if k < 12 else (1, 12)
                        return wh[h][:, (k - base) * 512:(k - base + 1) * 512]
                w8sl = w8j03[:, :, :] if j == 0 else w8r3[:, :, (j - 1) * 512: j * 512]
                w8bsl = w8bj03[:, :, :] if j == 0 else w8br3[:, :, (j - 1) * 512: j * 512]
                # (k15, B) DR operands: lhsT planes (0, 1+grp) of pair8,
                # rhs planes (j, 6+bslot) of b8all — bslot ordering makes the
                # plane stride 6 for every j
                psl = (pair83[:, 0:2, :] if grp == 0
                       else pair83[:, 0:3:2, :])
                b8sl = b8al3[:, j:j + 7:6, :]
                for i in range(NA if j == 0 else 0, NT):
                    ops = opsum.tile([P, 512], F32, tag="ops", name="ops")
                    if j == OJ - 1 and i == NT - 1:
                        # last tile as two 256-wide half-chains so the final
                        # copy+store drain starts ~1.5us earlier
                        osb = opool.tile([P, 512], BF16, tag="osb", name="osb")
                        for hh in range(2):
                            sl = slice(hh * 256, (hh + 1) * 256)
                            nc.tensor.matmul(
                                ops[:, sl], lhsT=x83[:, :, i * P:(i + 1) * P],
                                rhs=w8sl[:, :, sl],
                                start=True, stop=False, perf_mode=DR)
                            nc.tensor.matmul(
                                ops[:, sl], lhsT=x8b3[:, :, i * P:(i + 1) * P],
                                rhs=w8bsl[:, :, sl],
                                start=False, stop=False, perf_mode=DR)
                            for k in range(4, KC - 1):
                                nc.tensor.matmul(
                                    ops[:, sl],
                                    lhsT=xsb[:, k * T_CORE + i * P:
                                             k * T_CORE + (i + 1) * P],
                                    rhs=wslice(k)[:, sl],
                                    start=False, stop=False)
                            nc.tensor.matmul(
                                ops[:, sl], lhsT=psl[:, :, i * P:(i + 1) * P],
                                rhs=b8sl[:, :, sl],
                                start=False, stop=True, perf_mode=DR)
                            nc.scalar.copy(osb[:, sl], ops[:, sl])
                            eng = nc.sync if hh else nc.scalar
                            eng.dma_start(
                                y[i * P:(i + 1) * P,
                                  j * 512 + hh * 256: j * 512 + (hh + 1) * 256],
                                osb[:, sl])
                        continue
                    nc.tensor.matmul(
                        ops[:], lhsT=x83[:, :, i * P:(i + 1) * P], rhs=w8sl,
                        start=True, stop=False, perf_mode=DR)
                    nc.tensor.matmul(
                        ops[:], lhsT=x8b3[:, :, i * P:(i + 1) * P], rhs=w8bsl,
                        start=False, stop=False, perf_mode=DR)
                    for k in range(4, KC - 1):
                        nc.tensor.matmul(
                            ops[:],
                            lhsT=xsb[:, k * T_CORE + i * P: k * T_CORE + (i + 1) * P],
                            rhs=wslice(k),
                            start=False, stop=False)
                    if j == 0 and i == NA:
                        # stage-A chains' closing DRs + drains, emitted here so
                        # the DVE had chain-i=NA's 14 MMs of slack to build hm8
                        for ia in range(NA):
                            nc.tensor.matmul(
                                opsA[ia][:],
                                lhsT=pair83[:, 0:2, ia * P:(ia + 1) * P],
                                rhs=b8al3[:, 0:7:6, :],
                                start=False, stop=True, perf_mode=DR)
                            store(ia, 0, opsA[ia])
                    nc.tensor.matmul(
                        ops[:], lhsT=psl[:, :, i * P:(i + 1) * P], rhs=b8sl,
                        start=False, stop=True, perf_mode=DR)
                    store(i, j, ops)
    nc.compile()
    return nc


def prep_in_maps(x, weight, lora_A, lora_B_q, lora_B_k, lora_B_v,
                 lora_scaling, token_to_slot):
    bf = ml_dtypes.bfloat16
    x = np.asarray(x, dtype=np.float32)
    lora_A = np.asarray(lora_A, dtype=np.float32)
    Bg = [np.asarray(b, dtype=np.float32) for b in (lora_B_q, lora_B_k, lora_B_v)]
    sc = np.asarray(lora_scaling, dtype=np.float32)
    slot = np.asarray(token_to_slot).astype(np.int64)

    perm = np.argsort(slot, kind="stable")
    slot_s = slot[perm]

    wT = np.ascontiguousarray(weight.T.astype(bf))      # (2048, 3072)
    f8 = ml_dtypes.float8_e4m3

    def q8(a, s):
        return np.clip(np.asarray(a, np.float32) * s, -240, 240).astype(f8)

    # fp8 copies of hidden dims 0:256 (k-chunks 0-1) for DoubleRow,
    # plane-major [128, 2, n]: product-preserving scales x*S8, W/A /S8
    w83 = q8(np.asarray(weight, np.float32).T[0:256], 1.0 / S8).reshape(
        2, P, OUT).transpose(1, 0, 2)
    w8j0 = np.ascontiguousarray(w83[:, :, 0:512]).reshape(P, -1)
    w8r = np.ascontiguousarray(w83[:, :, 512:]).reshape(P, -1)
    w8c15 = q8(np.asarray(weight, np.float32).T[15 * P:16 * P], 1.0 / S8)
    w8b3 = q8(np.asarray(weight, np.float32).T[256:512], 1.0 / S8).reshape(
        2, P, OUT).transpose(1, 0, 2)
    w8bj0 = np.ascontiguousarray(w8b3[:, :, 0:512]).reshape(P, -1)
    w8br = np.ascontiguousarray(w8b3[:, :, 512:]).reshape(P, -1)

    in_maps = []
    for c in range(N_CORES):
        win = slice(c * T_CORE, (c + 1) * T_CORE)
        toks = perm[win]
        sl = slot_s[win]
        sids = np.unique(sl)
        if len(sids) > NSLOT:
            raise ValueError(f"core {c}: {len(sids)} slots > {NSLOT}")
        sids = np.concatenate([sids, -np.ones(NSLOT - len(sids), np.int64)])

        xf = x[toks].T                                    # (2048, 1024) fp32
        xTc = np.ascontiguousarray(xf.astype(bf))
        x8c = np.ascontiguousarray(
            q8(xf[0:256], S8).reshape(2, P, T_CORE).transpose(1, 0, 2)
        ).reshape(P, -1)
        x8c15 = np.ascontiguousarray(q8(xf[15 * P:16 * P], S8))
        x8b = np.ascontiguousarray(
            q8(xf[256:512], S8).reshape(2, P, T_CORE).transpose(1, 0, 2)
        ).reshape(P, -1)
        # packed rank layout: row g*64 + ls*16 + r for g in {0,1} -> group 1,
        # g=2 at rows 128:192 of group 2, rows 192:256 zero padding.
        a_l = np.zeros((GRP, HIDDEN), np.float32)
        b_l = [np.zeros((P, s), np.float32) for s in (Q_SIZE, KV_SIZE, KV_SIZE)]
        maskTc = np.zeros((GRP, T_CORE), np.float32)  # cast to bf16 on ship-out
        for ls, sid in enumerate(sids):
            if sid < 0:
                continue
            hit = (sl == sid).astype(np.float32)          # (1024,)
            for g in range(3):
                row = g * LR + ls * RANK                  # 0:192 packed
                a_l[row:row + RANK] = lora_A[sid, g]
                maskTc[row:row + RANK] = hit
            b_l[0][ls * RANK:(ls + 1) * RANK] = sc[sid] * Bg[0][sid].T   # g0 -> rows 0:64
            b_l[1][LR + ls * RANK: LR + (ls + 1) * RANK] = sc[sid] * Bg[1][sid].T  # g1 -> 64:128
            b_l[2][ls * RANK:(ls + 1) * RANK] = sc[sid] * Bg[2][sid].T   # g2 -> rows 0:64
        a8c = np.ascontiguousarray(
            q8(a_l[:, 0:256].T, 1.0 / S8).reshape(2, P, GRP).transpose(1, 0, 2)
        ).reshape(P, -1)
        b8 = np.ascontiguousarray(np.concatenate(
            [w8c15.astype(np.float32)] + [b_l[g] for g in range(3)],
            axis=1))
        in_maps.append({
            "xT": xTc,
            "x8d": x8c,
            "x8bd": x8b,
            "w8bj0d": w8bj0,
            "w8brd": w8br,
            "x8c15d": x8c15,
            "a8d": a8c,
            "w8j0d": w8j0,
            "w8rd": w8r,
            "b8alld": q8(b8, 1.0),
            "wT": wT,
            "aT": np.ascontiguousarray(a_l[0:192].T.astype(bf)),
            "maskT": q8(np.ascontiguousarray(maskTc), 1.0),
        })
    return in_maps, perm


# --- dense fallback (no token sorting) for pathological slot skew ---



def build_nc_dense():
    """Build the SPMD Bass program (same program on every core)."""
    nc = bacc.Bacc("TRN2", target_bir_lowering=False, debug=False, num_devices=N_CORES)

    xT = nc.dram_tensor("xT", [HIDDEN, T_CORE], BF16, kind="ExternalInput").ap()
    wT = nc.dram_tensor("wT", [HIDDEN, OUT], BF16, kind="ExternalInput").ap()
    aT = nc.dram_tensor("aT", [HIDDEN, 3 * GR], BF16, kind="ExternalInput").ap()
    bq = nc.dram_tensor("bq", [GR, Q_SIZE], BF16, kind="ExternalInput").ap()
    bk = nc.dram_tensor("bk", [GR, KV_SIZE], BF16, kind="ExternalInput").ap()
    bv = nc.dram_tensor("bv", [GR, KV_SIZE], BF16, kind="ExternalInput").ap()
    maskT = nc.dram_tensor("maskT", [GR, T_CORE], F32, kind="ExternalInput").ap()
    y = nc.dram_tensor("y", [T_CORE, OUT], F32, kind="ExternalOutput").ap()

    with tile.TileContext(nc) as tc:
        with (
            tc.tile_pool(name="xsb", bufs=1) as xpool,
            tc.tile_pool(name="asb", bufs=1) as apool,
            tc.tile_pool(name="bsb", bufs=1) as bpool,
            tc.tile_pool(name="msk", bufs=1) as mpool,
            tc.tile_pool(name="hm", bufs=1) as hmpool,
            tc.tile_pool(name="w", bufs=4) as wpool,
            tc.tile_pool(name="o", bufs=4) as opool,
            tc.tile_pool(name="hps", bufs=1, space="PSUM") as hpsum,
            tc.tile_pool(name="ops", bufs=4, space="PSUM") as opsum,
        ):
            xsb = xpool.tile([P, KC * T_CORE], BF16)   # free idx = k*T_CORE + t
            asb = apool.tile([P, KC * 3 * GR], BF16)   # free idx = k*384 + g*128+l*16+r
            # Each dma_start has ~0.6us fixed cost, so batch: A in 2 DMAs,
            # x in k-pair DMAs ordered by consumption (h0 pairs, then h1).
            # mask + B ride the scalar ring, which is idle until stores begin.
            aT3 = aT.rearrange("(c p) r -> p c r", p=P)
            xT3 = xT.rearrange("(c p) t -> p c t", p=P)
            asb3 = asb[:].rearrange("p (c r) -> p c r", c=KC)
            xsb3 = xsb[:].rearrange("p (c t) -> p c t", c=KC)
            # sync ring: pairwise (a, x-h0) paces phase-1 h0; then W follows.
            # scalar ring (idle until stores): x-h1, mask, B.
            for k in range(0, KC, 2):
                nc.sync.dma_start(asb3[:, k:k + 2, :], aT3[:, k:k + 2, :])
                nc.sync.dma_start(
                    xsb3[:, k:k + 2, 0:512], xT3[:, k:k + 2, 0:512])
            for k in range(0, KC, 2):
                nc.sync.dma_start(
                    xsb3[:, k:k + 2, 512:T_CORE], xT3[:, k:k + 2, 512:T_CORE])
            msk = mpool.tile([P, T_CORE], F32)
            nc.scalar.dma_start(msk[:], maskT[:, :])
            bqsb = bpool.tile([P, Q_SIZE], BF16)
            bksb = bpool.tile([P, KV_SIZE], BF16)
            bvsb = bpool.tile([P, KV_SIZE], BF16)
            nc.scalar.dma_start(bqsb[:], bq[:, :])
            nc.scalar.dma_start(bksb[:], bk[:, :])
            nc.scalar.dma_start(bvsb[:], bv[:, :])

            # --- phase 1: hT[g][gr, t] = sum_k A[g][gr, k] x[t, k], k outermost;
            # token halves sequential so only 3 PSUM banks are held ---
            hps = [hpsum.tile([P, 512], F32, tag=f"hps{c}", name=f"hps{c}")
                   for c in range(3)]
            hm = hmpool.tile([P, 3 * T_CORE], BF16)
            for hh in range(2):
                for k in range(KC):
                    for g in range(3):
                        nc.tensor.matmul(
                            hps[g][:],
                            lhsT=asb[:, k * 3 * GR + g * P: k * 3 * GR + (g + 1) * P],
                            rhs=xsb[:, k * T_CORE + hh * 512: k * T_CORE + (hh + 1) * 512],
                            start=(k == 0), stop=(k == KC - 1))
                # mask applied during PSUM drain; hm[g][gr, t] in bf16
                for g in range(3):
                    nc.vector.tensor_tensor(
                        hm[:, g * T_CORE + hh * 512: g * T_CORE + (hh + 1) * 512],
                        hps[g][:], msk[:, hh * 512:(hh + 1) * 512],
                        op=mybir.AluOpType.mult)

            # --- phase 2: y = x@W^T + hm@B, streamed over 512-wide o-chunks ---
            for j in range(OJ):
                if j < Q_SIZE // 512:
                    g, bsl = 0, bqsb[:, j * 512:(j + 1) * 512]
                elif j == Q_SIZE // 512:
                    g, bsl = 1, bksb[:]
                else:
                    g, bsl = 2, bvsb[:]
                wh = []
                for half in range(2):
                    wsb = wpool.tile([P, 8 * 512], BF16, tag="w", name="wsb")
                    # one batched DMA per half-tile: [128p, 8 chunks, 512]
                    nc.sync.dma_start(
                        wsb[:],
                        wT.rearrange("(c p) o -> p c o", p=P)[
                            :, half * 8:(half + 1) * 8, j * 512:(j + 1) * 512])
                    wh.append(wsb)
                for i in range(NT):
                    ops = opsum.tile([P, 512], F32, tag="ops", name="ops")
                    for k in range(KC):
                        nc.tensor.matmul(
                            ops[:],
                            lhsT=xsb[:, k * T_CORE + i * P: k * T_CORE + (i + 1) * P],
                            rhs=wh[k // 8][:, (k % 8) * 512:(k % 8 + 1) * 512],
                            start=(k == 0), stop=False)
                    nc.tensor.matmul(
                        ops[:],
                        lhsT=hm[:, g * T_CORE + i * P: g * T_CORE + (i + 1) * P],
                        rhs=bsl,
                        start=False, stop=True)
                    osb = opool.tile([P, 512], F32)
                    nc.scalar.copy(osb[:], ops[:])
                    # stores ride the scalar HWDGE ring, separate from W loads
                    nc.scalar.dma_start(
                        y[i * P:(i + 1) * P, j * 512:(j + 1) * 512], osb[:])
    nc.compile()
    return nc


def prep_in_maps_dense(x, weight, lora_A, lora_B_q, lora_B_k, lora_B_v,
                 lora_scaling, token_to_slot):
    bf = ml_dtypes.bfloat16
    x = np.asarray(x, dtype=np.float32)
    lora_scaling = np.asarray(lora_scaling, dtype=np.float32)
    slot = np.asarray(token_to_slot)

    xT = np.ascontiguousarray(np.asarray(x, dtype=np.float32).T.astype(bf))
    wT = np.ascontiguousarray(
        np.asarray(weight, dtype=np.float32).T.astype(bf))          # (2048, 3072)
    # aT col = g*128 + l*16 + r
    aT = np.ascontiguousarray(
        np.asarray(lora_A, dtype=np.float32)
        .transpose(1, 0, 2, 3).reshape(3 * GR, HIDDEN).T.astype(bf))
    # b row = l*16 + r, with scaling folded in
    sc = lora_scaling[:, None, None]
    bq = np.ascontiguousarray(
        (sc * np.asarray(lora_B_q, np.float32)).transpose(0, 2, 1)
        .reshape(GR, Q_SIZE).astype(bf))
    bk = np.ascontiguousarray(
        (sc * np.asarray(lora_B_k, np.float32)).transpose(0, 2, 1)
        .reshape(GR, KV_SIZE).astype(bf))
    bv = np.ascontiguousarray(
        (sc * np.asarray(lora_B_v, np.float32)).transpose(0, 2, 1)
        .reshape(GR, KV_SIZE).astype(bf))
    # one-hot routing mask, repeated over the 16 ranks: maskT[l*16+r, t]
    onehot = (np.arange(MAX_LORAS)[:, None] == slot[None, :]).astype(np.float32)
    maskT = np.repeat(onehot, RANK, axis=0)                         # (128, T)

    in_maps = []
    for c in range(N_CORES):
        sl = slice(c * T_CORE, (c + 1) * T_CORE)
        in_maps.append({
            "xT": np.ascontiguousarray(xT[:, sl]),
            "wT": wT,
            "aT": aT,
            "bq": bq,
            "bk": bk,
            "bv": bv,
            "maskT": np.ascontiguousarray(maskT[:, sl]),
        })
    return in_maps




def kernel(**inputs):
    from concourse.bass_utils import run_bass_kernel_spmd
    try:
        in_maps, perm = prep_in_maps(**inputs)
    except ValueError:
        # >NSLOT distinct slots in some sorted window: use the dense kernel
        if "ncd" not in _NC_CACHE:
            _NC_CACHE["ncd"] = build_nc_dense()
        in_maps = prep_in_maps_dense(**inputs)
        res = run_bass_kernel_spmd(_NC_CACHE["ncd"], in_maps,
                                   core_ids=list(range(N_CORES)))
        return np.concatenate([r["y"] for r in res.results], axis=0)
    if "nc" not in _NC_CACHE:
        _NC_CACHE["nc"] = build_nc()
    res = run_bass_kernel_spmd(_NC_CACHE["nc"], in_maps,
                               core_ids=list(range(N_CORES)))
    y_sorted = np.concatenate([r["y"] for r in res.results], axis=0)
    y = np.empty((T, OUT), np.float32)
    y[perm] = y_sorted.astype(np.float32)
    return y



# revision 26
# speedup vs baseline: 1.0857x; 1.0857x over previous
"""LoRA QKV with slot routing on 8 TRN2 cores — sorted-token sparse variant.

Host sorts tokens by LoRA slot, so each core's 1024-token shard touches at
most 4 slots (uniform routing over 8 slots; a 1024-token window of the sorted
order can't span more). Per-core LoRA state shrinks to 4 slots x 3 targets
x 16 ranks = 192 local ranks, padded to 256 so every matmul keeps K=M=128.
B matrices are zero-padded so each target group contracts over a full 128
partitions. The host un-permutes y afterwards.

Speed structure (HW-measured ~188us at 2.4GHz before the k2/k3 DR, vs
212us all-bf16; ~180us projected with it):
- fp8 DoubleRow: a DR matmul (2 fp8 k-planes) issues in one bf16 matmul's
  216ns slot -> each 14-instruction output chain covers 17 planes:
  DR(k0,k1) + DR(k2,k3) (e4m3, product-preserving scales x*S8 / W/S8)
  + 11 bf16 planes + DR(k15 | hm@B) where plane 1 pairs the on-device-
  quantized hm8 with B8. fp8 noise: 5 of 16 k-planes + hm/B at 1.69e-2
  rel err, hardware-verified and deterministic (gate 2e-2).
- stage A merges phase 1 (4 PSUM banks) with the first NA=4 j=0 chains
  (4 banks), k-paced on the input DMA stream, so the PE never idles long
  enough for the HAM clock gate to drop to 4/8.
- 10 warmup matmuls on a zeros tile bridge the HAM 1.2->2.4GHz ramp across
  the initial DMA-latency window.
- DMA: ~316GB/s/core aggregate, concurrent descriptors fair-share. Sync ring
  carries the k-paced (x, W-j0) stream; scalar carries a + stores. W j>=1
  halves are gated behind dummy w-pool generations (written after the mask
  multiply) so they cannot steal startup-window bandwidth; only k-chunks
  4..14 are fetched (0-3,15 ride fp8). Masks ship as fp8 (0/1 exact),
  y ships bf16 (host upcasts).
"""

import numpy as np
import ml_dtypes

import concourse.bass as bass
import concourse.bacc as bacc
import concourse.mybir as mybir
import concourse.tile as tile

HIDDEN = 2048
Q_SIZE = 2048
KV_SIZE = 512
OUT = Q_SIZE + 2 * KV_SIZE  # 3072
MAX_LORAS = 8
RANK = 16
T = 8192
N_CORES = 8
T_CORE = T // N_CORES  # 1024

P = 128
NT = T_CORE // P          # 8 token tiles per core
KC = HIDDEN // P          # 16 k-chunks
OJ = OUT // 512           # 6 output chunks of 512
NSLOT = 4                 # max distinct slots per sorted 1024-token window
LR = NSLOT * RANK         # 64 local ranks per target group
GRP = 2 * P               # 256 = [g0|g1|g2|zero-pad] packed as 2 groups of 128
GR = MAX_LORAS * RANK     # 128 (dense fallback rank count)
F32 = mybir.dt.float32
BF16 = mybir.dt.bfloat16
FP8 = mybir.dt.float8e4
FP8_NP = None  # ml_dtypes.float8_e4m3, resolved lazily
S8 = 0.25      # product-preserving fp8 scale: x*S8, (W,A)/S8

_NC_CACHE = {}


def build_nc():
    nc = bacc.Bacc("TRN2", target_bir_lowering=False, debug=False, num_devices=N_CORES)

    xT = nc.dram_tensor("xT", [HIDDEN, T_CORE], BF16, kind="ExternalInput").ap()
    wT = nc.dram_tensor("wT", [HIDDEN, OUT], BF16, kind="ExternalInput").ap()
    aT = nc.dram_tensor("aT", [HIDDEN, 192], BF16, kind="ExternalInput").ap()
    maskT = nc.dram_tensor("maskT", [GRP, T_CORE], FP8, kind="ExternalInput").ap()
    # fp8 (e4m3, TRN max-240 flavor) copies of k-chunks 0-1 for DoubleRow:
    # one DR matmul covers 2 k-planes in one bf16-matmul's issue slot
    # (216ns measured), saving one matmul per chain. Scales are product-
    # preserving (x*0.25, W*4) so DR output accumulates directly with the
    # bf16 chunks in PSUM.
    x8d = nc.dram_tensor("x8d", [P, 2 * T_CORE], FP8, kind="ExternalInput").ap()
    a8d = nc.dram_tensor("a8d", [P, 2 * GRP], FP8, kind="ExternalInput").ap()
    w8j0d = nc.dram_tensor("w8j0d", [P, 2 * 512], FP8, kind="ExternalInput").ap()
    w8rd = nc.dram_tensor("w8rd", [P, 2 * (OUT - 512)], FP8, kind="ExternalInput").ap()
    # second DR pair replaces (k15 bf16 MM + B MM): plane0 = x8ch15*w8ch15
    # (scales S8, 1/S8), plane1 = hm8*B8 (scales 1, 1) — per-plane products
    # are independently exact, so both accumulate into the same PSUM
    x8bd = nc.dram_tensor("x8bd", [P, 2 * T_CORE], FP8, kind="ExternalInput").ap()
    w8bj0d = nc.dram_tensor("w8bj0d", [P, 2 * 512], FP8, kind="ExternalInput").ap()
    w8brd = nc.dram_tensor("w8brd", [P, 2 * (OUT - 512)], FP8, kind="ExternalInput").ap()
    x8c15d = nc.dram_tensor("x8c15d", [P, T_CORE], FP8, kind="ExternalInput").ap()
    b8alld = nc.dram_tensor("b8alld", [P, 2 * OUT], FP8, kind="ExternalInput").ap()
    # y ships bf16 (host upcasts): halves store traffic + tail drain
    y = nc.dram_tensor("y", [T_CORE, OUT], BF16, kind="ExternalOutput").ap()

    NA = 4  # j0 chains merged into the k-paced stage A (PSUM: 4 hp + 4 ops)

    with tile.TileContext(nc) as tc:
        with (
            tc.tile_pool(name="xsb", bufs=1) as xpool,
            tc.tile_pool(name="asb", bufs=1) as apool,
            tc.tile_pool(name="wj0", bufs=1) as wj0pool,
            tc.tile_pool(name="bsb", bufs=1) as bpool,
            tc.tile_pool(name="msk", bufs=1) as mpool,
            tc.tile_pool(name="hm", bufs=1) as hmpool,
            tc.tile_pool(name="wrm", bufs=1) as wrmpool,
            tc.tile_pool(name="w", bufs=4) as wpool,
            tc.tile_pool(name="o", bufs=4) as opool,
            tc.tile_pool(name="hps", bufs=1, space="PSUM") as hpsum,
            tc.tile_pool(name="ops", bufs=4, space="PSUM") as opsum,
        ):
            xsb = xpool.tile([P, KC * T_CORE], BF16)   # free idx = k*T_CORE + t
            asb = apool.tile([P, KC * GRP], BF16)      # free idx = k*256 + lr
            wj0 = wj0pool.tile([P, KC * 512], BF16)    # W j=0, free idx = k*512+o
            aT3 = aT.rearrange("(c p) r -> p c r", p=P)
            xT3 = xT.rearrange("(c p) t -> p c t", p=P)
            wT3 = wT.rearrange("(c p) o -> p c o", p=P)
            asb3 = asb[:].rearrange("p (c r) -> p c r", c=KC)
            xsb3 = xsb[:].rearrange("p (c t) -> p c t", c=KC)
            wj03 = wj0[:].rearrange("p (c o) -> p c o", c=KC)
            x8sb = xpool.tile([P, 2 * T_CORE], FP8, tag="x8", name="x8sb")
            a8sb = apool.tile([P, 2 * GRP], FP8, tag="a8", name="a8sb")
            w8j0sb = wj0pool.tile([P, 2 * 512], FP8, tag="w8j0", name="w8j0sb")
            w8rsb = wj0pool.tile([P, 2 * (OUT - 512)], FP8, tag="w8r", name="w8rsb")
            x8bsb = xpool.tile([P, 2 * T_CORE], FP8, tag="x8b", name="x8bsb")
            w8bj0sb = wj0pool.tile([P, 2 * 512], FP8, tag="w8bj0", name="w8bj0sb")
            w8brsb = wj0pool.tile([P, 2 * (OUT - 512)], FP8, tag="w8br", name="w8brsb")
            x8b3 = x8bsb[:].rearrange("p (c t) -> p c t", c=2)
            w8bj03 = w8bj0sb[:].rearrange("p (c o) -> p c o", c=2)
            w8br3 = w8brsb[:].rearrange("p (c o) -> p c o", c=2)
            x83 = x8sb[:].rearrange("p (c t) -> p c t", c=2)
            a83 = a8sb[:].rearrange("p (c r) -> p c r", c=2)
            w8j03 = w8j0sb[:].rearrange("p (c o) -> p c o", c=2)
            w8r3 = w8rsb[:].rearrange("p (c o) -> p c o", c=2)
            # pair8: [x8ch15 | hm8 g0 | hm8 g1]; lhsT planes (0, 1+grp)
            pair8 = hmpool.tile([P, 3 * T_CORE], FP8, tag="pair8", name="pair8")
            pair83 = pair8[:].rearrange("p (c t) -> p c t", c=3)
            # b8all: 12 planes of 512: [w8ch15 j=0..5 | bq8 x4 | bk8 | bv8];
            # rhs planes (j, 6+bslot(j)) — always stride 6
            b8all = bpool.tile([P, 2 * OUT], FP8, tag="b8all", name="b8all")
            b8al3 = b8all[:].rearrange("p (c o) -> p c o", c=12)
            DR = mybir.MatmulPerfMode.DoubleRow
            # PE warmup: the HAM clock gate passes 4/8 pulses until ~3.4us of
            # sustained activity, so burn the initial DMA wait on dummy
            # matmuls (zeros tile; PSUM bank = hp1[0]'s earlier generation).
            wrm = wrmpool.tile([P, 640], BF16)
            nc.vector.memset(wrm[:], 0.0)
            # rank-pad columns of a (192:256 per chunk) must be finite zeros:
            # they feed hp2 garbage otherwise, and NaN*0 = NaN through the
            # mask multiply
            nc.gpsimd.memset(asb3[:, :, 192:256], 0.0)
            wps = hpsum.tile([P, 512], F32, tag="hp10", name="warm")
            for _ in range(10):
                nc.tensor.matmul(wps[:], lhsT=wrm[:, 0:128], rhs=wrm[:, 128:640],
                                 start=True, stop=True)
            # sync ring: interleaved (x, W j0) chunk pairs — everything stage A
            # consumes, in consumption order. scalar ring: a chunks + masks +
            # B + W j1/j2 halves (the first wpool generations have no
            # pool-rotation gate, so routing them here keeps them off the
            # x-critical sync queue; j>=3 halves are gated late and ride sync).
            nc.sync.dma_start(x8sb[:], x8d[:, :])
            nc.sync.dma_start(w8j0sb[:], w8j0d[:, :])
            # x23 (bf16, consumed by p1 k=2 first) goes ahead of the second
            # DR pair's fp8 operands (consumed after p1 k=3)
            nc.sync.dma_start(xsb3[:, 2:4, :], xT3[:, 2:4, :])
            nc.sync.dma_start(x8bsb[:], x8bd[:, :])
            nc.sync.dma_start(w8bj0sb[:], w8bj0d[:, :])
            for k in range(4, KC, 2):
                nc.sync.dma_start(xsb3[:, k:k + 2, :], xT3[:, k:k + 2, :])
                if k < KC - 2:
                    nc.sync.dma_start(wj03[:, k:k + 2, :], wT3[:, k:k + 2, 0:512])
                else:
                    nc.sync.dma_start(wj03[:, k:k + 1, :], wT3[:, k:k + 1, 0:512])
            nc.scalar.dma_start(a8sb[:], a8d[:, :])
            for k in range(2, KC, 2):
                nc.scalar.dma_start(asb3[:, k:k + 2, 0:192], aT3[:, k:k + 2, :])
            # msk + B at the sync queue TAIL: the ring depth defers their
            # transfers until the stage-A inputs are mostly done (they're only
            # needed at ~36us; in the early soup they'd steal ~1.25MB of the
            # startup window's ~316GB/s)
            msk = mpool.tile([P, 2 * T_CORE], FP8)  # [:, 0:T]=grp1, [:, T:2T]=grp2
            nc.sync.dma_start(msk[:, 0:T_CORE], maskT[0:P, :])
            nc.sync.dma_start(msk[:, T_CORE:2 * T_CORE], maskT[P:GRP, :])
            nc.sync.dma_start(pair8[:, 0:T_CORE], x8c15d[:, :])
            nc.sync.dma_start(b8all[:], b8alld[:, :])
            nc.sync.dma_start(w8rsb[:], w8rd[:, :])
            nc.sync.dma_start(w8brsb[:], w8brd[:, :])

            # --- stage A, k-outer: phase-1 h chains (2 rank groups x 2 token
            # halves) + the first NA j=0 output chains. 7 MMs per chunk
            # (~1.5us) vs ~1.2us DMA per chunk: PE-bound, so the HAM clock
            # never drops mid-stream. ---
            hp1 = [hpsum.tile([P, 512], F32, tag=f"hp1{c}", name=f"hp1{c}")
                   for c in range(2)]
            hp2 = [hpsum.tile([P, 512], F32, tag=f"hp2{c}", name=f"hp2{c}")
                   for c in range(2)]
            opsA = [opsum.tile([P, 512], F32, tag="ops", name=f"opsA{c}")
                    for c in range(NA)]
            for grp, hp in ((0, hp1), (1, hp2)):
                for hh in range(2):
                    nc.tensor.matmul(
                        hp[hh][:], lhsT=a83[:, :, grp * P:(grp + 1) * P],
                        rhs=x83[:, :, hh * 512:(hh + 1) * 512],
                        start=True, stop=False, perf_mode=DR)
            for i in range(NA):
                nc.tensor.matmul(
                    opsA[i][:], lhsT=x83[:, :, i * P:(i + 1) * P],
                    rhs=w8j03[:, :, :],
                    start=True, stop=False, perf_mode=DR)
            for k in range(2, KC):
                for grp, hp in ((0, hp1), (1, hp2)):
                    lhsT = asb[:, k * GRP + grp * P: k * GRP + (grp + 1) * P]
                    for hh in range(2):
                        nc.tensor.matmul(
                            hp[hh][:], lhsT=lhsT,
                            rhs=xsb[:, k * T_CORE + hh * 512:
                                    k * T_CORE + (hh + 1) * 512],
                            start=False, stop=(k == KC - 1))
                if k == 3:
                    for i in range(NA):
                        nc.tensor.matmul(
                            opsA[i][:], lhsT=x8b3[:, :, i * P:(i + 1) * P],
                            rhs=w8bj03[:, :, :],
                            start=False, stop=False, perf_mode=DR)
                if k < 4 or k == KC - 1:
                    continue  # chunks 2,3 ride the DR; k15 pairs with B
                for i in range(NA):
                    nc.tensor.matmul(
                        opsA[i][:],
                        lhsT=xsb[:, k * T_CORE + i * P: k * T_CORE + (i + 1) * P],
                        rhs=wj03[:, k, :],
                        start=False, stop=False)
            for grp, hp in ((0, hp1), (1, hp2)):
                for hh in range(2):
                    nc.vector.tensor_tensor(
                        pair8[:, (1 + grp) * T_CORE + hh * 512:
                              (1 + grp) * T_CORE + (hh + 1) * 512],
                        hp[hh][:],
                        msk[:, grp * T_CORE + hh * 512: grp * T_CORE + (hh + 1) * 512],
                        op=mybir.AluOpType.mult)
            # Gate the W j>=1 fetches out of the startup window: burn the
            # first 4 "w"-pool generations on dummy tiles whose writer (a
            # 1-column DVE copy reading hm) only runs after the mask-mult, so
            # the real halves (gens 4+) cannot issue while stage A still
            # needs the DMA bandwidth. A 1MB descriptor only gets
            # ~70-100GB/s, but 4+ concurrent ones saturate ~316GB/s/core —
            # anything extra in the window starves the k-paced chunks.
            for g in range(4):
                wgate = wpool.tile([P, 8 * 512], BF16, tag="w", name="wgate")
                nc.vector.tensor_scalar_add(wgate[:, 0:1],
                                            pair8[:, T_CORE:T_CORE + 1], 0.0)

            def store(i, j, ops):
                osb = opool.tile([P, 512], BF16, tag="osb", name="osb")
                nc.scalar.copy(osb[:], ops[:])
                eng = nc.sync if (i + j) % 2 else nc.scalar
                eng.dma_start(
                    y[i * P:(i + 1) * P, j * 512:(j + 1) * 512], osb[:])

            # --- phase 2: y = x@W^T + hm@B, streamed over 512-wide o-chunks.
            # hmA rows: g0 ranks 0:64, g1 ranks 64:128 -> bq rows 64:128 are
            # zero and bk rows 0:64 are zero, so each B matmul contracts
            # K=128. j=0 finishes first: chain i=NA runs before the stage-A
            # B-matmuls so the DVE mask-multiply isn't on the PE critical
            # path. ---
            for j in range(OJ):
                grp = 1 if j == OJ - 1 else 0
                if j == 0:
                    def wslice(k):
                        return wj03[:, k, :]
                else:
                    wh = []
                    for half in range(2):
                        wsb = wpool.tile([P, 8 * 512], BF16, tag="w", name="wsb")
                        lo, hi = (4, 12) if half == 0 else (12, 15)
                        nc.sync.dma_start(
                            wsb[:, 0:(hi - lo) * 512],
                            wT3[:, lo:hi, j * 512:(j + 1) * 512])
                        wh.append(wsb)

                    def wslice(k, wh=wh):
                        h, base = (0, 4) if k < 12 else (1, 12)
                        return wh[h][:, (k - base) * 512:(k - base + 1) * 512]
                w8sl = w8j03[:, :, :] if j == 0 else w8r3[:, :, (j - 1) * 512: j * 512]
                w8bsl = w8bj03[:, :, :] if j == 0 else w8br3[:, :, (j - 1) * 512: j * 512]
                # (k15, B) DR operands: lhsT planes (0, 1+grp) of pair8,
                # rhs planes (j, 6+bslot) of b8all — bslot ordering makes the
                # plane stride 6 for every j
                psl = (pair83[:, 0:2, :] if grp == 0
                       else pair83[:, 0:3:2, :])
                b8sl = b8al3[:, j:j + 7:6, :]
                for i in range(NA if j == 0 else 0, NT):
                    ops = opsum.tile([P, 512], F32, tag="ops", name="ops")
                    if j == OJ - 1 and i == NT - 1:
                        # last tile as two 256-wide half-chains so the final
                        # copy+store drain starts ~1.5us earlier
                        osb = opool.tile([P, 512], BF16, tag="osb", name="osb")
                        for hh in range(2):
                            sl = slice(hh * 256, (hh + 1) * 256)
                            nc.tensor.matmul(
                                ops[:, sl], lhsT=x83[:, :, i * P:(i + 1) * P],
                                rhs=w8sl[:, :, sl],
                                start=True, stop=False, perf_mode=DR)
                            nc.tensor.matmul(
                                ops[:, sl], lhsT=x8b3[:, :, i * P:(i + 1) * P],
                                rhs=w8bsl[:, :, sl],
                                start=False, stop=False, perf_mode=DR)
                            for k in range(4, KC - 1):
                                nc.tensor.matmul(
                                    ops[:, sl],
                                    lhsT=xsb[:, k * T_CORE + i * P:
                                             k * T_CORE + (i + 1) * P],
                                    rhs=wslice(k)[:, sl],
                                    start=False, stop=False)
                            nc.tensor.matmul(
                                ops[:, sl], lhsT=psl[:, :, i * P:(i + 1) * P],
                                rhs=b8sl[:, :, sl],
                                start=False, stop=True, perf_mode=DR)
                            nc.scalar.copy(osb[:, sl], ops[:, sl])
                            eng = nc.sync if hh else nc.scalar
                            eng.dma_start(
                                y[i * P:(i + 1) * P,
                                  j * 512 + hh * 256: j * 512 + (hh + 1) * 256],
                                osb[:, sl])
                        continue
                    nc.tensor.matmul(
                        ops[:], lhsT=x83[:, :, i * P:(i + 1) * P], rhs=w8sl,
                        start=True, stop=False, perf_mode=DR)
                    nc.tensor.matmul(
                        ops[:], lhsT=x8b3[:, :, i * P:(i + 1) * P], rhs=w8bsl,
                        start=False, stop=False, perf_mode=DR)
                    for k in range(4, KC - 1):
                        nc.tensor.matmul(
                            ops[:],
                            lhsT=xsb[:, k * T_CORE + i * P: k * T_CORE + (i + 1) * P],
                            rhs=wslice(k),
                            start=False, stop=False)
                    if j == 0 and i == NA:
                        # stage-A chains' closing DRs + drains, emitted here so
                        # the DVE had chain-i=NA's 14 MMs of slack to build hm8
                        for ia in range(NA):
                            nc.tensor.matmul(
                                opsA[ia][:],
                                lhsT=pair83[:, 0:2, ia * P:(ia + 1) * P],
                                rhs=b8al3[:, 0:7:6, :],
                                start=False, stop=True, perf_mode=DR)
                            store(ia, 0, opsA[ia])
                    nc.tensor.matmul(
                        ops[:], lhsT=psl[:, :, i * P:(i + 1) * P], rhs=b8sl,
                        start=False, stop=True, perf_mode=DR)
                    store(i, j, ops)
    nc.compile()
    return nc


def prep_in_maps(x, weight, lora_A, lora_B_q, lora_B_k, lora_B_v,
                 lora_scaling, token_to_slot):
    bf = ml_dtypes.bfloat16
    x = np.asarray(x, dtype=np.float32)
    lora_A = np.asarray(lora_A, dtype=np.float32)
    Bg = [np.asarray(b, dtype=np.float32) for b in (lora_B_q, lora_B_k, lora_B_v)]
    sc = np.asarray(lora_scaling, dtype=np.float32)
    slot = np.asarray(token_to_slot).astype(np.int64)

    perm = np.argsort(slot, kind="stable")
    slot_s = slot[perm]

    wT = np.ascontiguousarray(weight.T.astype(bf))      # (2048, 3072)
    f8 = ml_dtypes.float8_e4m3

    def q8(a, s):
        return np.clip(np.asarray(a, np.float32) * s, -240, 240).astype(f8)

    # fp8 copies of hidden dims 0:256 (k-chunks 0-1) for DoubleRow,
    # plane-major [128, 2, n]: product-preserving scales x*S8, W/A /S8
    w83 = q8(np.asarray(weight, np.float32).T[0:256], 1.0 / S8).reshape(
        2, P, OUT).transpose(1, 0, 2)
    w8j0 = np.ascontiguousarray(w83[:, :, 0:512]).reshape(P, -1)
    w8r = np.ascontiguousarray(w83[:, :, 512:]).reshape(P, -1)
    w8c15 = q8(np.asarray(weight, np.float32).T[15 * P:16 * P], 1.0 / S8)
    w8b3 = q8(np.asarray(weight, np.float32).T[256:512], 1.0 / S8).reshape(
        2, P, OUT).transpose(1, 0, 2)
    w8bj0 = np.ascontiguousarray(w8b3[:, :, 0:512]).reshape(P, -1)
    w8br = np.ascontiguousarray(w8b3[:, :, 512:]).reshape(P, -1)

    in_maps = []
    for c in range(N_CORES):
        win = slice(c * T_CORE, (c + 1) * T_CORE)
        toks = perm[win]
        sl = slot_s[win]
        sids = np.unique(sl)
        if len(sids) > NSLOT:
            raise ValueError(f"core {c}: {len(sids)} slots > {NSLOT}")
        sids = np.concatenate([sids, -np.ones(NSLOT - len(sids), np.int64)])

        xf = x[toks].T                                    # (2048, 1024) fp32
        xTc = np.ascontiguousarray(xf.astype(bf))
        x8c = np.ascontiguousarray(
            q8(xf[0:256], S8).reshape(2, P, T_CORE).transpose(1, 0, 2)
        ).reshape(P, -1)
        x8c15 = np.ascontiguousarray(q8(xf[15 * P:16 * P], S8))
        x8b = np.ascontiguousarray(
            q8(xf[256:512], S8).reshape(2, P, T_CORE).transpose(1, 0, 2)
        ).reshape(P, -1)
        # packed rank layout: row g*64 + ls*16 + r for g in {0,1} -> group 1,
        # g=2 at rows 128:192 of group 2, rows 192:256 zero padding.
        a_l = np.zeros((GRP, HIDDEN), np.float32)
        b_l = [np.zeros((P, s), np.float32) for s in (Q_SIZE, KV_SIZE, KV_SIZE)]
        maskTc = np.zeros((GRP, T_CORE), np.float32)  # cast to bf16 on ship-out
        for ls, sid in enumerate(sids):
            if sid < 0:
                continue
            hit = (sl == sid).astype(np.float32)          # (1024,)
            for g in range(3):
                row = g * LR + ls * RANK                  # 0:192 packed
                a_l[row:row + RANK] = lora_A[sid, g]
                maskTc[row:row + RANK] = hit
            b_l[0][ls * RANK:(ls + 1) * RANK] = sc[sid] * Bg[0][sid].T   # g0 -> rows 0:64
            b_l[1][LR + ls * RANK: LR + (ls + 1) * RANK] = sc[sid] * Bg[1][sid].T  # g1 -> 64:128
            b_l[2][ls * RANK:(ls + 1) * RANK] = sc[sid] * Bg[2][sid].T   # g2 -> rows 0:64
        a8c = np.ascontiguousarray(
            q8(a_l[:, 0:256].T, 1.0 / S8).reshape(2, P, GRP).transpose(1, 0, 2)
        ).reshape(P, -1)
        b8 = np.ascontiguousarray(np.concatenate(
            [w8c15.astype(np.float32)] + [b_l[g] for g in range(3)],
            axis=1))
        in_maps.append({
            "xT": xTc,
            "x8d": x8c,
            "x8bd": x8b,
            "w8bj0d": w8bj0,
            "w8brd": w8br,
            "x8c15d": x8c15,
            "a8d": a8c,
            "w8j0d": w8j0,
            "w8rd": w8r,
            "b8alld": q8(b8, 1.0),
            "wT": wT,
            "aT": np.ascontiguousarray(a_l[0:192].T.astype(bf)),
            "maskT": q8(np.ascontiguousarray(maskTc), 1.0),
        })
    return in_maps, perm


# --- dense fallback (no token sorting) for pathological slot skew ---



def build_nc_dense():
    """Build the SPMD Bass program (same program on every core)."""
    nc = bacc.Bacc("TRN2", target_bir_lowering=False, debug=False, num_devices=N_CORES)

    xT = nc.dram_tensor("xT", [HIDDEN, T_CORE], BF16, kind="ExternalInput").ap()
    wT = nc.dram_tensor("wT", [HIDDEN, OUT], BF16, kind="ExternalInput").ap()
    aT = nc.dram_tensor("aT", [HIDDEN, 3 * GR], BF16, kind="ExternalInput").ap()
    bq = nc.dram_tensor("bq", [GR, Q_SIZE], BF16, kind="ExternalInput").ap()
    bk = nc.dram_tensor("bk", [GR, KV_SIZE], BF16, kind="ExternalInput").ap()
    bv = nc.dram_tensor("bv", [GR, KV_SIZE], BF16, kind="ExternalInput").ap()
    maskT = nc.dram_tensor("maskT", [GR, T_CORE], F32, kind="ExternalInput").ap()
    y = nc.dram_tensor("y", [T_CORE, OUT], F32, kind="ExternalOutput").ap()

    with tile.TileContext(nc) as tc:
        with (
            tc.tile_pool(name="xsb", bufs=1) as xpool,
            tc.tile_pool(name="asb", bufs=1) as apool,
            tc.tile_pool(name="bsb", bufs=1) as bpool,
            tc.tile_pool(name="msk", bufs=1) as mpool,
            tc.tile_pool(name="hm", bufs=1) as hmpool,
            tc.tile_pool(name="w", bufs=4) as wpool,
            tc.tile_pool(name="o", bufs=4) as opool,
            tc.tile_pool(name="hps", bufs=1, space="PSUM") as hpsum,
            tc.tile_pool(name="ops", bufs=4, space="PSUM") as opsum,
        ):
            xsb = xpool.tile([P, KC * T_CORE], BF16)   # free idx = k*T_CORE + t
            asb = apool.tile([P, KC * 3 * GR], BF16)   # free idx = k*384 + g*128+l*16+r
            # Each dma_start has ~0.6us fixed cost, so batch: A in 2 DMAs,
            # x in k-pair DMAs ordered by consumption (h0 pairs, then h1).
            # mask + B ride the scalar ring, which is idle until stores begin.
            aT3 = aT.rearrange("(c p) r -> p c r", p=P)
            xT3 = xT.rearrange("(c p) t -> p c t", p=P)
            asb3 = asb[:].rearrange("p (c r) -> p c r", c=KC)
            xsb3 = xsb[:].rearrange("p (c t) -> p c t", c=KC)
            # sync ring: pairwise (a, x-h0) paces phase-1 h0; then W follows.
            # scalar ring (idle until stores): x-h1, mask, B.
            for k in range(0, KC, 2):
                nc.sync.dma_start(asb3[:, k:k + 2, :], aT3[:, k:k + 2, :])
                nc.sync.dma_start(
                    xsb3[:, k:k + 2, 0:512], xT3[:, k:k + 2, 0:512])
            for k in range(0, KC, 2):
                nc.sync.dma_start(
                    xsb3[:, k:k + 2, 512:T_CORE], xT3[:, k:k + 2, 512:T_CORE])
            msk = mpool.tile([P, T_CORE], F32)
            nc.scalar.dma_start(msk[:], maskT[:, :])
            bqsb = bpool.tile([P, Q_SIZE], BF16)
            bksb = bpool.tile([P, KV_SIZE], BF16)
            bvsb = bpool.tile([P, KV_SIZE], BF16)
            nc.scalar.dma_start(bqsb[:], bq[:, :])
            nc.scalar.dma_start(bksb[:], bk[:, :])
            nc.scalar.dma_start(bvsb[:], bv[:, :])

            # --- phase 1: hT[g][gr, t] = sum_k A[g][gr, k] x[t, k], k outermost;
            # token halves sequential so only 3 PSUM banks are held ---
            hps = [hpsum.tile([P, 512], F32, tag=f"hps{c}", name=f"hps{c}")
                   for c in range(3)]
            hm = hmpool.tile([P, 3 * T_CORE], BF16)
            for hh in range(2):
                for k in range(KC):
                    for g in range(3):
                        nc.tensor.matmul(
                            hps[g][:],
                            lhsT=asb[:, k * 3 * GR + g * P: k * 3 * GR + (g + 1) * P],
                            rhs=xsb[:, k * T_CORE + hh * 512: k * T_CORE + (hh + 1) * 512],
                            start=(k == 0), stop=(k == KC - 1))
                # mask applied during PSUM drain; hm[g][gr, t] in bf16
                for g in range(3):
                    nc.vector.tensor_tensor(
                        hm[:, g * T_CORE + hh * 512: g * T_CORE + (hh + 1) * 512],
                        hps[g][:], msk[:, hh * 512:(hh + 1) * 512],
                        op=mybir.AluOpType.mult)

            # --- phase 2: y = x@W^T + hm@B, streamed over 512-wide o-chunks ---
            for j in range(OJ):
                if j < Q_SIZE // 512:
                    g, bsl = 0, bqsb[:, j * 512:(j + 1) * 512]
                elif j == Q_SIZE // 512:
                    g, bsl = 1, bksb[:]
                else:
                    g, bsl = 2, bvsb[:]
                wh = []
                for half in range(2):
                    wsb = wpool.tile([P, 8 * 512], BF16, tag="w", name="wsb")
                    # one batched DMA per half-tile: [128p, 8 chunks, 512]
                    nc.sync.dma_start(
                        wsb[:],
                        wT.rearrange("(c p) o -> p c o", p=P)[
                            :, half * 8:(half + 1) * 8, j * 512:(j + 1) * 512])
                    wh.append(wsb)
                for i in range(NT):
                    ops = opsum.tile([P, 512], F32, tag="ops", name="ops")
                    for k in range(KC):
                        nc.tensor.matmul(
                            ops[:],
                            lhsT=xsb[:, k * T_CORE + i * P: k * T_CORE + (i + 1) * P],
                            rhs=wh[k // 8][:, (k % 8) * 512:(k % 8 + 1) * 512],
                            start=(k == 0), stop=False)
                    nc.tensor.matmul(
                        ops[:],
                        lhsT=hm[:, g * T_CORE + i * P: g * T_CORE + (i + 1) * P],
                        rhs=bsl,
                        start=False, stop=True)
                    osb = opool.tile([P, 512], F32)
                    nc.scalar.copy(osb[:], ops[:])
                    # stores ride the scalar HWDGE ring, separate from W loads
                    nc.scalar.dma_start(
                        y[i * P:(i + 1) * P, j * 512:(j + 1) * 512], osb[:])
    nc.compile()
    return nc


def prep_in_maps_dense(x, weight, lora_A, lora_B_q, lora_B_k, lora_B_v,
                 lora_scaling, token_to_slot):
    bf = ml_dtypes.bfloat16
    x = np.asarray(x, dtype=np.float32)
    lora_scaling = np.asarray(lora_scaling, dtype=np.float32)
    slot = np.asarray(token_to_slot)

    xT = np.ascontiguousarray(np.asarray(x, dtype=np.float32).T.astype(bf))
    wT = np.ascontiguousarray(
        np.asarray(weight, dtype=np.float32).T.astype(bf))          # (2048, 3072)
    # aT col = g*128 + l*16 + r
    aT = np.ascontiguousarray(
        np.asarray(lora_A, dtype=np.float32)
        .transpose(1, 0, 2, 3).reshape(3 * GR, HIDDEN).T.astype(bf))
    # b row = l*16 + r, with scaling folded in
    sc = lora_scaling[:, None, None]
    bq = np.ascontiguousarray(
        (sc * np.asarray(lora_B_q, np.float32)).transpose(0, 2, 1)
        .reshape(GR, Q_SIZE).astype(bf))
    bk = np.ascontiguousarray(
        (sc * np.asarray(lora_B_k, np.float32)).transpose(0, 2, 1)
        .reshape(GR, KV_SIZE).astype(bf))
    bv = np.ascontiguousarray(
        (sc * np.asarray(lora_B_v, np.float32)).transpose(0, 2, 1)
        .reshape(GR, KV_SIZE).astype(bf))
    # one-hot routing mask, repeated over the 16 ranks: maskT[l*16+r, t]
    onehot = (np.arange(MAX_LORAS)[:, None] == slot[None, :]).astype(np.float32)
    maskT = np.repeat(onehot, RANK, axis=0)                         # (128, T)

    in_maps = []
    for c in range(N_CORES):
        sl = slice(c * T_CORE, (c + 1) * T_CORE)
        in_maps.append({
            "xT": np.ascontiguousarray(xT[:, sl]),
            "wT": wT,
            "aT": aT,
            "bq": bq,
            "bk": bk,
            "bv": bv,
            "maskT": np.ascontiguousarray(maskT[:, sl]),
        })
    return in_maps




def kernel(**inputs):
    from concourse.bass_utils import run_bass_kernel_spmd
    try:
        in_maps, perm = prep_in_maps(**inputs)
    except ValueError:
        # >NSLOT distinct slots in some sorted window: use the dense kernel
        if "ncd" not in _NC_CACHE:
            _NC_CACHE["ncd"] = build_nc_dense()
        in_maps = prep_in_maps_dense(**inputs)
        res = run_bass_kernel_spmd(_NC_CACHE["ncd"], in_maps,
                                   core_ids=list(range(N_CORES)))
        return np.concatenate([r["y"] for r in res.results], axis=0)
    if "nc" not in _NC_CACHE:
        _NC_CACHE["nc"] = build_nc()
    res = run_bass_kernel_spmd(_NC_CACHE["nc"], in_maps,
                               core_ids=list(range(N_CORES)))
    y_sorted = np.concatenate([r["y"] for r in res.results], axis=0)
    y = np.empty((T, OUT), np.float32)
    y[perm] = y_sorted.astype(np.float32)
    return y

